# revision 1
# baseline (speedup 1.0000x reference)
"""Multi-head causal attention Bass kernel for Trainium2, 8-core SPMD.

Problem: B=2, S=2048, D=1024, H=16, DH=64.
  q = x @ Wq; k = x @ Wk; v = x @ Wv  (per head h: 64-wide column slices)
  out = softmax(causal(q k^T / 8)) v

Sharding: core c -> batch b = c // 4, head group g = c % 4 (heads 4g..4g+3).
Each core gets x[b]^T (transposed on host) and 256-wide W column slices,
computes 4 heads over the full sequence, returns y [2, 128, 2048] =
ctx^T stacked per head pair. Host reassembles/transposes.

Per-core layout (matmul operands float32r = full-rate, ~tf32 accuracy):
  xT_ch[ch] [128, 8*512]   ch = s-chunk; k-chunk kk at cols [512*kk, ...)
  w*_all    [128, 8*256]   k-chunk kk at cols [256*kk, ...)
  QT/KT     per (m, ch) tiles [128, 512] (rows = W cols j; head h at
                                          tile h//2, partitions (h%2)*64)
  V4[q]     [128, 4*260]   s-tiles 4q..4q+3; within a 260-block: per head
                           64 V cols + 1 ones col (softmax denominator row)
  scores^T per (head, i-chunk of 512) in j-groups of 2 tiles:
      psum [128, 1024] -> exp via ACT (scale=1/8) -> SBUF f32r
      causal diagonal blocks masked via gpsimd affine_select (fill 0)
  ctx^T[e, i] accumulated in psum [65, 512] over j-tiles; row 64 = denom l[i]
  normalize: l -> SBUF -> DVE recip_approx -> gpsimd partition_broadcast ->
  DVE mul -> ctx_sb -> DMA out
"""

import sys

import numpy as np

try:
    import concourse.bass as bass  # noqa: F401
except ImportError:
    for _p in ("/opt/trn_rl_repo", "/root/.axon_site/_ro/trn_rl_repo"):
        if _p not in sys.path:
            sys.path.insert(0, _p)
    import concourse.bass as bass  # noqa: F401

from concourse import bacc
import concourse.mybir as mybir
import concourse.tile as tile

F32 = mybir.dt.float32
F32R = mybir.dt.float32r

S = 2048          # sequence length
D = 1024          # model dim (contraction for projections)
HPC = 4           # heads per core
DH = 64           # head dim
NK = D // 128     # 8 contraction chunks
NST = S // 128    # 16 sequence tiles of 128
NCH = S // 512    # 4 s-chunks of 512
VW = HPC * (DH + 1)   # 260 cols per V s-tile


def build_kernel(loop_n=0):
    nc = bacc.Bacc("TRN2", target_bir_lowering=False, debug=True)

    xT = nc.dram_tensor("xT", [D, S], F32R, kind="ExternalInput")
    wq = nc.dram_tensor("wq", [D, HPC * DH], F32R, kind="ExternalInput")
    wk = nc.dram_tensor("wk", [D, HPC * DH], F32R, kind="ExternalInput")
    wv = nc.dram_tensor("wv", [D, HPC * DH], F32R, kind="ExternalInput")
    ones_in = nc.dram_tensor("ones_in", [128, 4, HPC, 1], F32R,
                             kind="ExternalInput")
    y = nc.dram_tensor("y", [2, 128, S], F32, kind="ExternalOutput")

    with tile.TileContext(nc) as tc:
        from contextlib import ExitStack
        stk = ExitStack()
        loop = stk.enter_context(tc.For_i(0, loop_n, 1)) if loop_n else None
        with stk, (
            tc.tile_pool(name="persist", bufs=1)
        ) as pers, (
            tc.tile_pool(name="proj_ps", bufs=2, space="PSUM")
        ) as proj_ps, (
            tc.tile_pool(name="score_ps", bufs=2, space="PSUM")
        ) as score_ps, (
            tc.tile_pool(name="ctx_ps", bufs=2, space="PSUM")
        ) as ctx_ps_pool, (
            tc.tile_pool(name="esb", bufs=4)
        ) as esb_pool, (
            tc.tile_pool(name="norm", bufs=2)
        ) as norm_pool:
            # ---- persistent SBUF tiles -------------------------------------
            xT_ch = [
                pers.tile([128, NK * 512], F32R, tag=f"xTc{ch}", name=f"xTc{ch}")
                for ch in range(NCH)
            ]
            w_all = {
                wname: pers.tile([128, NK * HPC * DH], F32R, name=f"w_{wname}")
                for wname in ("q", "k", "v")
            }
            QT_sb = [
                [pers.tile([128, 512], F32R, tag=f"QT{m}c{ch}",
                           name=f"QT{m}c{ch}") for ch in range(NCH)]
                for m in range(2)
            ]
            KT_sb = [
                [pers.tile([128, 512], F32R, tag=f"KT{m}c{ch}",
                           name=f"KT{m}c{ch}") for ch in range(NCH)]
                for m in range(2)
            ]
            V4 = [
                pers.tile([128, 4 * VW], F32R, tag=f"V4_{q}", name=f"V4_{q}")
                for q in range(4)
            ]
            ctx_sb = [
                pers.tile([128, S], F32, tag=f"ctx{p}", name=f"ctx{p}")
                for p in range(2)
            ]

            def xs(ch, kk):      # xT chunk ch, k-chunk kk -> [128, 512]
                return xT_ch[ch][:, 512 * kk:512 * (kk + 1)]

            def ws(wname, kk):   # w k-chunk [128, 256]
                return w_all[wname][:, HPC * DH * kk:HPC * DH * (kk + 1)]

            def vs(t):           # V s-tile t -> [128, 260]
                return V4[t // 4][:, VW * (t % 4):VW * (t % 4 + 1)]

            # ---- input DMAs ------------------------------------------------
            # W first (small, needed by every projection) on the SP HWDGE ring;
            # xT s-chunks on the ACT HWDGE ring so they stream in parallel.
            for wname, wdram in (("q", wq), ("k", wk), ("v", wv)):
                nc.sync.dma_start(
                    out=w_all[wname].rearrange("p (k e) -> p k e", k=NK),
                    in_=wdram.rearrange("(k p) e -> p k e", k=NK),
                )
            xTr = xT.rearrange("(k p) (c s) -> p c k s", k=NK, c=NCH)
            for ch in range(NCH):
                nc.scalar.dma_start(
                    out=xT_ch[ch].rearrange("p (k s) -> p k s", k=NK),
                    in_=xTr[:, ch],
                )
            # ones columns of V (denominator rows): one strided DMA per V4
            for q in range(4):
                nc.sync.dma_start(
                    out=V4[q].rearrange("p (t h c) -> p t h c", t=4, h=HPC)[
                        :, :, :, DH:DH + 1
                    ],
                    in_=ones_in[:],
                )

            # ---- projections (emitted per s-chunk, interleaved with
            # attention: attention i-chunk c needs only chunks <= c) --------
            def emit_proj_chunk(ch):
                # Q^T/K^T: out[j, s] = sum_d W[d, j] * xT[d, s]
                for m in range(2):
                    for wname, dest in (("q", QT_sb), ("k", KT_sb)):
                        ps = proj_ps.tile([128, 512], F32, tag="proj", name="ps_qk")
                        for kk in range(NK):
                            nc.tensor.matmul(
                                ps[:],
                                ws(wname, kk)[:, 128 * m:128 * (m + 1)],
                                xs(ch, kk),
                                start=(kk == 0),
                                stop=(kk == NK - 1),
                            )
                        nc.vector.tensor_copy(dest[m][ch][:], ps[:])
                # V: out[s, e] = sum_d xT[d, s] * Wv[d, e]
                for t in range(4 * ch, 4 * ch + 4):
                    ps = proj_ps.tile([128, HPC * DH], F32, tag="proj", name="ps_v")
                    for kk in range(NK):
                        nc.tensor.matmul(
                            ps[:],
                            xs(t // 4, kk)[:, 128 * (t % 4):128 * (t % 4 + 1)],
                            ws("v", kk),
                            start=(kk == 0),
                            stop=(kk == NK - 1),
                        )
                    nc.vector.tensor_copy(
                        vs(t).rearrange("p (h c) -> p h c", h=HPC)[:, :, 0:DH],
                        ps.rearrange("p (h c) -> p h c", h=HPC),
                    )

            # ---- attention -----------------------------------------
            # Head-PAIR packed scores: for pair p = h//2, one psum
            # [128, 1024] holds head A (cols 0:512) and head B
            # (cols 512:1024) scores^T for ONE j-tile, computed by two
            # row-strip-packed K=64 matmuls that run concurrently on the
            # PE sub-arrays. One exp covers both heads. ctx accumulates
            # per head in its own [65, 512] psum (ones row = denom).
            e_sbs = {}

            def emit_scores(key):
                pair, c, jt = key
                m = pair
                ps = score_ps.tile([128, 1024], F32, tag="score",
                                   name="s_ps")
                for half in range(2):
                    off = half * 64
                    nc.tensor.matmul(
                        ps[:, 512 * half:512 * (half + 1)],
                        KT_sb[m][jt // 4][off:off + 64,
                                          128 * (jt % 4):128 * (jt % 4 + 1)],
                        QT_sb[m][c][off:off + 64, :],
                        start=True,
                        stop=True,
                        tile_position=(off, 0),
                    )
                e = esb_pool.tile([128, 1024], F32R, tag="esb", name="e_sb")
                nc.scalar.activation(
                    out=e[:], in_=ps[:],
                    func=mybir.ActivationFunctionType.Exp, scale=0.125,
                )
                if jt >= 4 * c:
                    # diagonal j-tile: keep where di - dj - o >= 0
                    o = (jt - 4 * c) * 128
                    e3 = e.rearrange("p (h i) -> p h i", h=2)
                    nc.gpsimd.affine_select(
                        e3,
                        e3,
                        pattern=[[0, 2], [1, 512]],
                        compare_op=mybir.AluOpType.is_ge,
                        fill=0.0,
                        base=-o,
                        channel_multiplier=-1,
                    )
                e_sbs[key] = e

            def finish_ctx(ctx_psum, h, c):
                pair, off = h // 2, (h % 2) * 64
                lrow = norm_pool.tile([1, 512], F32, tag="lrow", name="lrow")
                nc.vector.tensor_copy(lrow[:], ctx_psum[64:65, :])
                recip = norm_pool.tile([1, 512], F32, tag="recip",
                                       name="recip")
                nc.vector.reciprocal_approx_fast(out=recip[:], in_=lrow[:])
                bc = norm_pool.tile([64, 512], F32, tag="bc", name="bc")
                nc.gpsimd.partition_broadcast(bc[:], recip[:])
                nc.vector.tensor_mul(
                    ctx_sb[pair][off:off + 64, 512 * c:512 * (c + 1)],
                    ctx_psum[0:64, :],
                    bc[:],
                )
                if h % 2 == 1:
                    nc.sync.dma_start(
                        out=y[pair, :, 512 * c:512 * (c + 1)],
                        in_=ctx_sb[pair][:, 512 * c:512 * (c + 1)],
                    )

            # finish chains are emitted one pair LATE so their gpsimd
            # partition_broadcast queues behind the next pair's first causal
            # masks instead of ahead of them (gpsimd executes in FIFO order)
            pending_finish = []

            def flush_finish():
                while pending_finish:
                    args = pending_finish.pop(0)
                    finish_ctx(*args)

            for c in range(NCH):
                emit_proj_chunk(c)
                njt = 4 * (c + 1)
                for pair in range(2):
                    keys = [(pair, c, jt) for jt in range(njt)]
                    emit_scores(keys[0])
                    flush_finish()
                    ctxA = ctx_ps_pool.tile([65, 512], F32, tag="ctx",
                                            name="ctx_psA")
                    ctxB = ctx_ps_pool.tile([65, 512], F32, tag="ctx",
                                            name="ctx_psB")
                    for idx, key in enumerate(keys):
                        if idx + 1 < len(keys):
                            emit_scores(keys[idx + 1])
                        _, _, jt = key
                        e = e_sbs.pop(key)
                        for half, cps in ((0, ctxA), (1, ctxB)):
                            h = 2 * pair + half
                            nc.tensor.matmul(
                                cps[:],
                                vs(jt)[:, (DH + 1) * h:(DH + 1) * (h + 1)],
                                e[:, 512 * half:512 * (half + 1)],
                                start=(idx == 0),
                                stop=(idx == njt - 1),
                            )
                    pending_finish.append((ctxA, 2 * pair, c))
                    pending_finish.append((ctxB, 2 * pair + 1, c))
            flush_finish()
    nc.compile()
    return nc


_CACHED = None


def get_nc():
    global _CACHED
    if _CACHED is None:
        _CACHED = build_kernel()
    return _CACHED


def shard_inputs(x, W_query, W_key, W_value):
    """Full inputs -> per-core input maps."""
    in_maps = []
    ones = np.ones((128, 4, HPC, 1), np.float32)
    # one transpose per batch, shared by the 4 cores of that batch
    xT_by_batch = [np.ascontiguousarray(x[b].T) for b in range(2)]
    for core in range(8):
        b, g = core // 4, core % 4
        sl = slice(256 * g, 256 * (g + 1))
        in_maps.append({
            "xT": xT_by_batch[b],
            "wq": np.ascontiguousarray(W_query[:, sl]),
            "wk": np.ascontiguousarray(W_key[:, sl]),
            "wv": np.ascontiguousarray(W_value[:, sl]),
            "ones_in": ones,
        })
    return in_maps


def assemble_output(results):
    """Per-core y [2, 128, S] -> full [2, S, 1024]."""
    out = np.empty((2, S, 1024), np.float32)
    for core in range(8):
        b, g = core // 4, core % 4
        yv = results[core]["y"]  # [2, 128, S]
        blk = yv.reshape(2, 2, 64, S).transpose(3, 0, 1, 2).reshape(S, 256)
        out[b, :, 256 * g:256 * (g + 1)] = blk
    return out


def kernel(x, W_query, W_key, W_value):
    """Full inputs in, full output out; 8-core SPMD underneath."""
    from concourse.bass_utils import run_bass_kernel_spmd

    x = np.ascontiguousarray(np.asarray(x, dtype=np.float32))
    W_query = np.ascontiguousarray(np.asarray(W_query, dtype=np.float32))
    W_key = np.ascontiguousarray(np.asarray(W_key, dtype=np.float32))
    W_value = np.ascontiguousarray(np.asarray(W_value, dtype=np.float32))

    nc = get_nc()
    in_maps = shard_inputs(x, W_query, W_key, W_value)
    last_err = None
    for _attempt in range(3):
        try:
            res = run_bass_kernel_spmd(nc, in_maps, core_ids=list(range(8)))
            return assemble_output(res.results)
        except Exception as e:  # transient device wedges seen on this fabric
            last_err = e
            import time as _time
            _time.sleep(2.0)
    raise last_err



# revision 2
# speedup vs baseline: 1.3601x; 1.3601x over previous
"""Multi-head causal attention Bass kernel for Trainium2, 8-core SPMD.

Problem: B=2, S=2048, D=1024, H=16, DH=64.
  q = x @ Wq; k = x @ Wk; v = x @ Wv  (per head h: 64-wide column slices)
  out = softmax(causal(q k^T / 8)) v

Sharding: core c -> batch b = c // 4, head group g = c % 4 (heads 4g..4g+3).
Each core gets x[b]^T (transposed on host, bf16) and 256-wide W column
slices (bf16), computes 4 heads over the full sequence, returns
y [2, 128, 2048] = ctx^T stacked per head pair. Host reassembles.

Per-core layout (matmul operands bf16 = full-rate):
  xT_ch[ch] [128, 8*512]   ch = s-chunk; k-chunk kk at cols [512*kk, ...)
  w*_all    [128, 8*256]   k-chunk kk at cols [256*kk, ...)
  QT/KT     per (m, ch) tiles [128, 512] bf16 (rows = W cols j; head h at
                                          tile h//2, partitions (h%2)*64)
  V4[q]     [128, 4*260]   bf16; s-tiles 4q..4q+3; per head 64 V cols +
                           1 ones col (softmax denominator row)
  scores^T per (head-pair, i-chunk c, j-tile jt): psum [128, 1024]
      (head A cols 0:512, head B 512:1024). Causal diagonal j-tiles are
      column-trimmed: only i in [128*(jt-4c), 512) is computed/exped/
      accumulated; the 128-wide causal boundary block is masked by a
      DVE multiply with a precomputed 0/1 mask (mask2 input).
  ctx^T[e, i] accumulated in psum [65, 512] over j-tiles; row 64 = denom
  normalize: l -> SBUF -> DVE recip_approx -> gpsimd partition_broadcast ->
  DVE mul -> ctx_sb -> DMA out

Input DMAs: W/ones/mask on SP ring; xT chunk 0 on ACT ring; xT chunks
1-3 on Pool ring (keeps the exp-heavy ACT engine free after startup).
"""

import sys

import numpy as np

try:
    import concourse.bass as bass  # noqa: F401
except ImportError:
    for _p in ("/opt/trn_rl_repo", "/root/.axon_site/_ro/trn_rl_repo"):
        if _p not in sys.path:
            sys.path.insert(0, _p)
    import concourse.bass as bass  # noqa: F401

from concourse import bacc
import concourse.mybir as mybir
import concourse.tile as tile

F32 = mybir.dt.float32
BF16 = mybir.dt.bfloat16

S = 2048          # sequence length
D = 1024          # model dim (contraction for projections)
HPC = 4           # heads per core
DH = 64           # head dim
NK = D // 128     # 8 contraction chunks
NST = S // 128    # 16 sequence tiles of 128
NCH = S // 512    # 4 s-chunks of 512
VW = HPC * (DH + 1)   # 260 cols per V s-tile


def build_kernel(loop_n=0):
    nc = bacc.Bacc("TRN2", target_bir_lowering=False, debug=True)

    xT = nc.dram_tensor("xT", [D, S], BF16, kind="ExternalInput")
    wq = nc.dram_tensor("wq", [D, HPC * DH], BF16, kind="ExternalInput")
    wk = nc.dram_tensor("wk", [D, HPC * DH], BF16, kind="ExternalInput")
    wv = nc.dram_tensor("wv", [D, HPC * DH], BF16, kind="ExternalInput")
    ones_in = nc.dram_tensor("ones_in", [128, 4, HPC, 1], BF16,
                             kind="ExternalInput")
    mask_in = nc.dram_tensor("mask_in", [128, 512], BF16,
                             kind="ExternalInput")
    y = nc.dram_tensor("y", [2, 128, S], F32, kind="ExternalOutput")

    with tile.TileContext(nc) as tc:
        from contextlib import ExitStack
        stk = ExitStack()
        loop = stk.enter_context(tc.For_i(0, loop_n, 1)) if loop_n else None
        with stk, (
            tc.tile_pool(name="persist", bufs=1)
        ) as pers, (
            tc.tile_pool(name="proj_ps", bufs=2, space="PSUM")
        ) as proj_ps, (
            tc.tile_pool(name="score_ps", bufs=2, space="PSUM")
        ) as score_ps, (
            tc.tile_pool(name="ctx_ps", bufs=2, space="PSUM")
        ) as ctx_ps_pool, (
            tc.tile_pool(name="esb", bufs=4)
        ) as esb_pool, (
            tc.tile_pool(name="norm", bufs=2)
        ) as norm_pool:
            # ---- persistent SBUF tiles -------------------------------------
            xT_ch = [
                pers.tile([128, NK * 512], BF16, tag=f"xTc{ch}", name=f"xTc{ch}")
                for ch in range(NCH)
            ]
            w_all = {
                wname: pers.tile([128, NK * HPC * DH], BF16, name=f"w_{wname}")
                for wname in ("q", "k", "v")
            }
            QT_sb = [
                [pers.tile([128, 512], BF16, tag=f"QT{m}c{ch}",
                           name=f"QT{m}c{ch}") for ch in range(NCH)]
                for m in range(2)
            ]
            KT_sb = [
                [pers.tile([128, 512], BF16, tag=f"KT{m}c{ch}",
                           name=f"KT{m}c{ch}") for ch in range(NCH)]
                for m in range(2)
            ]
            V4 = [
                pers.tile([128, 4 * VW], BF16, tag=f"V4_{q}", name=f"V4_{q}")
                for q in range(4)
            ]
            ctx_sb = [
                pers.tile([128, S], F32, tag=f"ctx{p}", name=f"ctx{p}")
                for p in range(2)
            ]
            mask2 = pers.tile([128, 512], BF16, tag="mask2", name="mask2")

            def xs(ch, kk):      # xT chunk ch, k-chunk kk -> [128, 512]
                return xT_ch[ch][:, 512 * kk:512 * (kk + 1)]

            def ws(wname, kk):   # w k-chunk [128, 256]
                return w_all[wname][:, HPC * DH * kk:HPC * DH * (kk + 1)]

            def vs(t):           # V s-tile t -> [128, 260]
                return V4[t // 4][:, VW * (t % 4):VW * (t % 4 + 1)]

            # ---- input DMAs ------------------------------------------------
            # W + ones + mask (small, needed by every projection) on the SP
            # HWDGE ring; xT s-chunk 0 on the ACT ring (ACT is idle until the
            # first scores land ~8us in); chunks 1-3 on the Pool ring.
            for wname, wdram in (("q", wq), ("k", wk), ("v", wv)):
                nc.sync.dma_start(
                    out=w_all[wname].rearrange("p (k e) -> p k e", k=NK),
                    in_=wdram.rearrange("(k p) e -> p k e", k=NK),
                )
            xTr = xT.rearrange("(k p) (c s) -> p c k s", k=NK, c=NCH)
            nc.scalar.dma_start(
                out=xT_ch[0].rearrange("p (k s) -> p k s", k=NK),
                in_=xTr[:, 0],
            )
            for ch in range(1, NCH):
                nc.gpsimd.dma_start(
                    out=xT_ch[ch].rearrange("p (k s) -> p k s", k=NK),
                    in_=xTr[:, ch],
                )
            # ones columns of V (denominator rows): one strided DMA per V4
            for q in range(4):
                nc.sync.dma_start(
                    out=V4[q].rearrange("p (t h c) -> p t h c", t=4, h=HPC)[
                        :, :, :, DH:DH + 1
                    ],
                    in_=ones_in[:],
                )
            nc.sync.dma_start(out=mask2[:], in_=mask_in[:])
            mask3 = mask2.rearrange("p (h i) -> p h i", h=2)

            # ---- projections (emitted per s-chunk, interleaved with
            # attention: attention i-chunk c needs only chunks <= c) --------
            def emit_proj_chunk(ch):
                # Q^T/K^T: out[j, s] = sum_d W[d, j] * xT[d, s]
                for m in range(2):
                    for wname, dest in (("q", QT_sb), ("k", KT_sb)):
                        ps = proj_ps.tile([128, 512], F32, tag="proj", name="ps_qk")
                        for kk in range(NK):
                            nc.tensor.matmul(
                                ps[:],
                                ws(wname, kk)[:, 128 * m:128 * (m + 1)],
                                xs(ch, kk),
                                start=(kk == 0),
                                stop=(kk == NK - 1),
                            )
                        nc.vector.tensor_copy(dest[m][ch][:], ps[:])
                # V: out[s, e] = sum_d xT[d, s] * Wv[d, e]
                for t in range(4 * ch, 4 * ch + 4):
                    ps = proj_ps.tile([128, HPC * DH], F32, tag="proj", name="ps_v")
                    for kk in range(NK):
                        nc.tensor.matmul(
                            ps[:],
                            xs(t // 4, kk)[:, 128 * (t % 4):128 * (t % 4 + 1)],
                            ws("v", kk),
                            start=(kk == 0),
                            stop=(kk == NK - 1),
                        )
                    nc.vector.tensor_copy(
                        vs(t).rearrange("p (h c) -> p h c", h=HPC)[:, :, 0:DH],
                        ps.rearrange("p (h c) -> p h c", h=HPC),
                    )

            # ---- attention -----------------------------------------
            # Head-PAIR packed scores: for pair p = h//2, one psum
            # [128, 1024] holds head A (cols 0:512) and head B
            # (cols 512:1024) scores^T for ONE j-tile, via two
            # row-strip-packed K=64 matmuls. One exp covers both heads.
            # Diagonal j-tiles (jt >= 4c, r = jt-4c) are trimmed to
            # i in [128r, 512); the causal boundary is the 128-wide block
            # [128r, 128r+128) masked by mask2 (i'' >= dj pattern).
            e_sbs = {}

            def lo_of(c, jt):
                r = jt - 4 * c
                return 128 * r if r > 0 else 0

            def emit_scores(key):
                pair, c, jt = key
                m = pair
                lo = lo_of(c, jt)
                w = 512 - lo
                ps = score_ps.tile([128, 1024], F32, tag="score",
                                   name="s_ps")
                ps3 = ps.rearrange("p (h i) -> p h i", h=2)
                for half in range(2):
                    off = half * 64
                    nc.tensor.matmul(
                        ps[:, 512 * half + lo:512 * (half + 1)],
                        KT_sb[m][jt // 4][off:off + 64,
                                          128 * (jt % 4):128 * (jt % 4 + 1)],
                        QT_sb[m][c][off:off + 64, lo:512],
                        start=True,
                        stop=True,
                        tile_position=(off, 0),
                    )
                e = esb_pool.tile([128, 1024], BF16, tag="esb", name="e_sb")
                e3 = e.rearrange("p (h i) -> p h i", h=2)
                nc.scalar.activation(
                    out=e3[:, :, lo:512], in_=ps3[:, :, lo:512],
                    func=mybir.ActivationFunctionType.Exp, scale=0.125,
                )
                if jt >= 4 * c:
                    # causal boundary block: multiply by the 0/1 mask
                    # (keep where i'' >= dj), i'' local to [lo, lo+128)
                    nc.vector.tensor_mul(
                        e3[:, :, lo:lo + 128],
                        e3[:, :, lo:lo + 128],
                        mask3[:, :, 128:256],
                    )
                e_sbs[key] = e

            def finish_ctx(ctx_psum, h, c):
                pair, off = h // 2, (h % 2) * 64
                lrow = norm_pool.tile([1, 512], F32, tag="lrow", name="lrow")
                nc.vector.tensor_copy(lrow[:], ctx_psum[64:65, :])
                recip = norm_pool.tile([1, 512], F32, tag="recip",
                                       name="recip")
                nc.vector.reciprocal_approx_fast(out=recip[:], in_=lrow[:])
                bc = norm_pool.tile([64, 512], F32, tag="bc", name="bc")
                nc.gpsimd.partition_broadcast(bc[:], recip[:])
                nc.vector.tensor_mul(
                    ctx_sb[pair][off:off + 64, 512 * c:512 * (c + 1)],
                    ctx_psum[0:64, :],
                    bc[:],
                )
                if h % 2 == 1:
                    nc.sync.dma_start(
                        out=y[pair, :, 512 * c:512 * (c + 1)],
                        in_=ctx_sb[pair][:, 512 * c:512 * (c + 1)],
                    )

            # finish chains are emitted one pair LATE so their gpsimd
            # partition_broadcast queues behind the Pool-ring input DMAs
            # instead of stalling the attention pipeline
            pending_finish = []

            def flush_finish():
                while pending_finish:
                    args = pending_finish.pop(0)
                    finish_ctx(*args)

            for c in range(NCH):
                emit_proj_chunk(c)
                njt = 4 * (c + 1)
                for pair in range(2):
                    keys = [(pair, c, jt) for jt in range(njt)]
                    emit_scores(keys[0])
                    flush_finish()
                    ctxA = ctx_ps_pool.tile([65, 512], F32, tag="ctx",
                                            name="ctx_psA")
                    ctxB = ctx_ps_pool.tile([65, 512], F32, tag="ctx",
                                            name="ctx_psB")
                    for idx, key in enumerate(keys):
                        if idx + 1 < len(keys):
                            emit_scores(keys[idx + 1])
                        _, _, jt = key
                        lo = lo_of(c, jt)
                        e = e_sbs.pop(key)
                        for half, cps in ((0, ctxA), (1, ctxB)):
                            h = 2 * pair + half
                            nc.tensor.matmul(
                                cps[:, lo:512],
                                vs(jt)[:, (DH + 1) * h:(DH + 1) * (h + 1)],
                                e[:, 512 * half + lo:512 * (half + 1)],
                                start=(idx == 0),
                                stop=(idx == njt - 1),
                            )
                    pending_finish.append((ctxA, 2 * pair, c))
                    pending_finish.append((ctxB, 2 * pair + 1, c))
            flush_finish()
    nc.compile()
    return nc


_CACHED = None


def get_nc():
    global _CACHED
    if _CACHED is None:
        _CACHED = build_kernel()
    return _CACHED


def _make_mask():
    """[128, 512] bf16: cols [128,256) and [384,512) hold the boundary
    mask M[dj, i''] = 1 if i'' >= dj (keep at-or-below diagonal)."""
    import ml_dtypes
    m = np.zeros((128, 512), np.float32)
    tri = np.triu(np.ones((128, 128), np.float32))  # tri[dj, i''] = i'' >= dj
    m[:, 128:256] = tri
    m[:, 384:512] = tri
    return m.astype(ml_dtypes.bfloat16)


def shard_inputs(x, W_query, W_key, W_value):
    """Full inputs -> per-core input maps (bf16 on the wire)."""
    import ml_dtypes
    bf = ml_dtypes.bfloat16
    in_maps = []
    ones = np.ones((128, 4, HPC, 1), bf)
    mask = _make_mask()
    # one transpose per batch, shared by the 4 cores of that batch
    xT_by_batch = [np.ascontiguousarray(x[b].T.astype(bf)) for b in range(2)]
    Wq16 = W_query.astype(bf)
    Wk16 = W_key.astype(bf)
    Wv16 = W_value.astype(bf)
    for core in range(8):
        b, g = core // 4, core % 4
        sl = slice(256 * g, 256 * (g + 1))
        in_maps.append({
            "xT": xT_by_batch[b],
            "wq": np.ascontiguousarray(Wq16[:, sl]),
            "wk": np.ascontiguousarray(Wk16[:, sl]),
            "wv": np.ascontiguousarray(Wv16[:, sl]),
            "ones_in": ones,
            "mask_in": mask,
        })
    return in_maps


def assemble_output(results):
    """Per-core y [2, 128, S] -> full [2, S, 1024]."""
    out = np.empty((2, S, 1024), np.float32)
    for core in range(8):
        b, g = core // 4, core % 4
        yv = results[core]["y"]  # [2, 128, S]
        blk = yv.reshape(2, 2, 64, S).transpose(3, 0, 1, 2).reshape(S, 256)
        out[b, :, 256 * g:256 * (g + 1)] = blk
    return out


def kernel(x, W_query, W_key, W_value):
    """Full inputs in, full output out; 8-core SPMD underneath."""
    from concourse.bass_utils import run_bass_kernel_spmd

    x = np.ascontiguousarray(np.asarray(x, dtype=np.float32))
    W_query = np.ascontiguousarray(np.asarray(W_query, dtype=np.float32))
    W_key = np.ascontiguousarray(np.asarray(W_key, dtype=np.float32))
    W_value = np.ascontiguousarray(np.asarray(W_value, dtype=np.float32))

    nc = get_nc()
    in_maps = shard_inputs(x, W_query, W_key, W_value)
    last_err = None
    for _attempt in range(3):
        try:
            res = run_bass_kernel_spmd(nc, in_maps, core_ids=list(range(8)))
            return assemble_output(res.results)
        except Exception as e:  # transient device wedges seen on this fabric
            last_err = e
            import time as _time
            _time.sleep(2.0)
    raise last_err


# revision 4
# speedup vs baseline: 1.8391x; 1.3523x over previous
"""Multi-head causal attention Bass kernel for Trainium2, 8-core SPMD.

Problem: B=2, S=2048, D=1024, H=16, DH=64.
  q = x @ Wq; k = x @ Wk; v = x @ Wv  (per head h: 64-wide column slices)
  out = softmax(causal(q k^T / 8)) v

Sharding: core c -> batch b = c // 4, head group g = c % 4 (heads 4g..4g+3).
Each core gets x[b]^T (transposed on host, bf16) and 256-wide W column
slices (bf16), computes 4 heads over the full sequence, returns
y [2, 128, 2048] = ctx^T stacked per head pair. Host reassembles.

Per-core layout (matmul operands bf16 = full-rate):
  xT_ch[ch] [128, 8*512]   ch = s-chunk; k-chunk kk at cols [512*kk, ...)
  w*_all    [128, 8*256]   k-chunk kk at cols [256*kk, ...)
  QT/KT     per (m, ch) tiles [128, 512] bf16 (rows = W cols j; head h at
                                          tile h//2, partitions (h%2)*64)
  V4[q]     [128, 4*260]   bf16; s-tiles 4q..4q+3; per head 64 V cols +
                           1 ones col (softmax denominator row)
  scores^T per (head-pair, i-chunk c, j-tile jt): psum [128, 1024]
      (head A cols 0:512, head B 512:1024). Causal diagonal j-tiles are
      column-trimmed: only i in [128*(jt-4c), 512) is computed/exped/
      accumulated; the 128-wide causal boundary block is masked by a
      DVE multiply with a precomputed 0/1 mask (mask2 input).
  ctx in [i-partitions, e-free] layout: stationary = e slice, moving =
  V [128 j, 65] (64 V cols + ones col -> denominator l in col 64); psum
  zero-regions are bank-sized so each ctx bank takes ONE start/stop and
  relies on pending-zero first-touch for its 8 accumulation regions.
  normalize: per-partition DVE reciprocal of l + per-partition-scalar
  multiplies into natural [s, e] output staging (y [16, 128, 256]).

Input DMAs: W/ones/mask on SP ring; xT chunk 0 on ACT ring; xT chunks
1-3 on Pool ring (keeps the exp-heavy ACT engine free after startup).
"""

import sys

import numpy as np

try:
    import concourse.bass as bass  # noqa: F401
except ImportError:
    for _p in ("/opt/trn_rl_repo", "/root/.axon_site/_ro/trn_rl_repo"):
        if _p not in sys.path:
            sys.path.insert(0, _p)
    import concourse.bass as bass  # noqa: F401

from concourse import bacc
import concourse.mybir as mybir
import concourse.tile as tile

F32 = mybir.dt.float32
BF16 = mybir.dt.bfloat16

S = 2048          # sequence length
D = 1024          # model dim (contraction for projections)
HPC = 4           # heads per core
DH = 64           # head dim
NK = D // 128     # 8 contraction chunks
NST = S // 128    # 16 sequence tiles of 128
NCH = S // 512    # 4 s-chunks of 512
VW = HPC * (DH + 1)   # 260 cols per V s-tile


def build_kernel(loop_n=0):
    nc = bacc.Bacc("TRN2", target_bir_lowering=False, debug=True)

    xT = nc.dram_tensor("xT", [D, S], BF16, kind="ExternalInput")
    wq = nc.dram_tensor("wq", [D, HPC * DH], BF16, kind="ExternalInput")
    wk = nc.dram_tensor("wk", [D, HPC * DH], BF16, kind="ExternalInput")
    wv = nc.dram_tensor("wv", [D, HPC * DH], BF16, kind="ExternalInput")
    ones_in = nc.dram_tensor("ones_in", [128, 4, HPC, 1], BF16,
                             kind="ExternalInput")
    mask_in = nc.dram_tensor("mask_in", [128, 512], BF16,
                             kind="ExternalInput")
    # natural [s, e] layout: s-tile t, s within tile, 4 heads x 64
    y = nc.dram_tensor("y", [NST, 128, HPC * DH], F32, kind="ExternalOutput")

    with tile.TileContext(nc) as tc:
        from contextlib import ExitStack
        stk = ExitStack()
        loop = stk.enter_context(tc.For_i(0, loop_n, 1)) if loop_n else None
        with stk, (
            tc.tile_pool(name="persist", bufs=1)
        ) as pers, (
            tc.tile_pool(name="proj_qk_ps", bufs=1, space="PSUM")
        ) as proj_qk_ps, (
            tc.tile_pool(name="proj_v_ps", bufs=1, space="PSUM")
        ) as proj_v_ps, (
            tc.tile_pool(name="score_ps", bufs=2, space="PSUM")
        ) as score_ps, (
            tc.tile_pool(name="ctx_ps", bufs=2, space="PSUM")
        ) as ctx_ps_pool, (
            tc.tile_pool(name="esb", bufs=4)
        ) as esb_pool, (
            tc.tile_pool(name="norm", bufs=2)
        ) as norm_pool:
            # ---- persistent SBUF tiles -------------------------------------
            xT_ch = [
                pers.tile([128, NK * 512], BF16, tag=f"xTc{ch}", name=f"xTc{ch}")
                for ch in range(NCH)
            ]
            w_all = {
                wname: pers.tile([128, NK * HPC * DH], BF16, name=f"w_{wname}")
                for wname in ("q", "k", "v")
            }
            QT_sb = [
                [pers.tile([128, 512], BF16, tag=f"QT{m}c{ch}",
                           name=f"QT{m}c{ch}") for ch in range(NCH)]
                for m in range(2)
            ]
            KT_sb = [
                [pers.tile([128, 512], BF16, tag=f"KT{m}c{ch}",
                           name=f"KT{m}c{ch}") for ch in range(NCH)]
                for m in range(2)
            ]
            V4 = [
                pers.tile([128, 4 * VW], BF16, tag=f"V4_{q}", name=f"V4_{q}")
                for q in range(4)
            ]
            # output staging, natural [s, e] layout: s-tile t at cols 256*t
            y_sb = pers.tile([128, NST * HPC * DH], F32, tag="y_sb",
                             name="y_sb")
            mask2 = pers.tile([128, 512], BF16, tag="mask2", name="mask2")

            def xs(ch, kk):      # xT chunk ch, k-chunk kk -> [128, 512]
                return xT_ch[ch][:, 512 * kk:512 * (kk + 1)]

            def ws(wname, kk):   # w k-chunk [128, 256]
                return w_all[wname][:, HPC * DH * kk:HPC * DH * (kk + 1)]

            def vs(t):           # V s-tile t -> [128, 260]
                return V4[t // 4][:, VW * (t % 4):VW * (t % 4 + 1)]

            # ---- input DMAs ------------------------------------------------
            # Startup critical path: first proj matmul needs wq chunk 0 +
            # xTc0 chunk 0.  Spread the first-needed tensors across rings:
            # wq on SP, wk on DVE, xTc0 split in two halves on ACT (so the
            # first half lands early); wv/ones/mask follow on SP; xT chunks
            # 1-3 go on the Pool ring.
            wr = {n: d.rearrange("(k p) e -> p k e", k=NK)
                  for n, d in (("q", wq), ("k", wk), ("v", wv))}
            w_sb = {n: w_all[n].rearrange("p (k e) -> p k e", k=NK)
                    for n in ("q", "k", "v")}
            nc.sync.dma_start(out=w_sb["q"], in_=wr["q"])
            nc.gpsimd.dma_start(out=w_sb["k"], in_=wr["k"])
            xTr = xT.rearrange("(k p) (c s) -> p c k s", k=NK, c=NCH)
            xT0r = xT_ch[0].rearrange("p (k s) -> p k s", k=NK)
            nc.scalar.dma_start(out=xT0r[:, 0:4], in_=xTr[:, 0, 0:4])
            nc.scalar.dma_start(out=xT0r[:, 4:8], in_=xTr[:, 0, 4:8])
            nc.sync.dma_start(out=w_sb["v"], in_=wr["v"])
            for ch in range(1, NCH):
                nc.gpsimd.dma_start(
                    out=xT_ch[ch].rearrange("p (k s) -> p k s", k=NK),
                    in_=xTr[:, ch],
                )
            # ones columns of V (denominator rows): one strided DMA per V4
            for q in range(4):
                nc.sync.dma_start(
                    out=V4[q].rearrange("p (t h c) -> p t h c", t=4, h=HPC)[
                        :, :, :, DH:DH + 1
                    ],
                    in_=ones_in[:],
                )
            nc.sync.dma_start(out=mask2[:], in_=mask_in[:])
            mask3 = mask2.rearrange("p (h i) -> p h i", h=2)

            # ---- projections (emitted per s-chunk, interleaved with
            # attention: attention i-chunk c needs only chunks <= c) --------
            def emit_proj_qk(ch, m, wname, dest, cp):
                # Q^T/K^T: out[j, s] = sum_d W[d, j] * xT[d, s]
                ps = proj_qk_ps.tile([128, 512], F32, tag="pqk", name="ps_qk")
                for kk in range(NK):
                    nc.tensor.matmul(
                        ps[:],
                        ws(wname, kk)[:, 128 * m:128 * (m + 1)],
                        xs(ch, kk),
                        start=(kk == 0),
                        stop=(kk == NK - 1),
                    )
                cp.tensor_copy(dest[m][ch][:], ps[:])

            def emit_proj_v(t, cp):
                # V: out[s, e] = sum_d xT[d, s] * Wv[d, e]
                ps = proj_v_ps.tile([128, HPC * DH], F32, tag="pv", name="ps_v")
                for kk in range(NK):
                    nc.tensor.matmul(
                        ps[:],
                        xs(t // 4, kk)[:, 128 * (t % 4):128 * (t % 4 + 1)],
                        ws("v", kk),
                        start=(kk == 0),
                        stop=(kk == NK - 1),
                    )
                cp.tensor_copy(
                    vs(t).rearrange("p (h c) -> p h c", h=HPC)[:, :, 0:DH],
                    ps.rearrange("p (h c) -> p h c", h=HPC),
                )

            def proj_units(ch):
                """One thunk per proj psum-group for s-chunk ch.  QK and V
                groups alternate so each group's psum copy drains behind the
                other pool's matmuls (both proj pools are single-buffered).
                psum->SBUF copies: DVE for chunk 0 (Pool's queue is still
                draining the xT chunk 1-3 input DMAs then), Pool afterwards
                (keeps DVE free for the causal-mask muls on the exp->ctx
                critical path)."""
                cp = nc.vector  # GPSIMD cannot access PSUM on real hw
                units = []
                order = [("qk", 0, "q", QT_sb), ("v", 4 * ch),
                         ("qk", 0, "k", KT_sb), ("v", 4 * ch + 1),
                         ("qk", 1, "q", QT_sb), ("v", 4 * ch + 2),
                         ("qk", 1, "k", KT_sb), ("v", 4 * ch + 3)]
                for item in order:
                    if item[0] == "qk":
                        units.append(lambda it=item: emit_proj_qk(
                            ch, it[1], it[2], it[3], cp))
                    else:
                        units.append(lambda it=item: emit_proj_v(it[1], cp))
                return units

            # ---- attention -----------------------------------------
            # Head-PAIR packed scores: for pair p = h//2, one psum
            # [128, 1024] holds head A (cols 0:512) and head B
            # (cols 512:1024) scores^T for ONE j-tile, via two
            # row-strip-packed K=64 matmuls. One exp covers both heads.
            # Diagonal j-tiles (jt >= 4c, r = jt-4c) are trimmed to
            # i in [128r, 512); the causal boundary is the 128-wide block
            # [128r, 128r+128) masked by mask2 (i'' >= dj pattern).
            e_sbs = {}

            def emit_scores(key):
                pair, c, jt = key
                m = pair
                r = jt - 4 * c
                lo = 128 * r if r > 0 else 0
                ps = score_ps.tile([128, 1024], F32, tag="score",
                                   name="s_ps")
                ps3 = ps.rearrange("p (h i) -> p h i", h=2)
                for half in range(2):
                    off = half * 64
                    nc.tensor.matmul(
                        ps[:, 512 * half + lo:512 * (half + 1)],
                        KT_sb[m][jt // 4][off:off + 64,
                                          128 * (jt % 4):128 * (jt % 4 + 1)],
                        QT_sb[m][c][off:off + 64, lo:512],
                        start=True,
                        stop=True,
                        tile_position=(off, 0),
                    )
                e = esb_pool.tile([128, 1024], BF16, tag="esb", name="e_sb")
                e3 = e.rearrange("p (h i) -> p h i", h=2)
                nc.scalar.activation(
                    out=e3[:, :, lo:512], in_=ps3[:, :, lo:512],
                    func=mybir.ActivationFunctionType.Exp, scale=0.125,
                )
                if r >= 0:
                    # causal boundary block: multiply by the 0/1 mask
                    # (keep where i'' >= dj), i'' local to [lo, lo+128)
                    nc.vector.tensor_mul(
                        e3[:, :, lo:lo + 128],
                        e3[:, :, lo:lo + 128],
                        mask3[:, :, 128:256],
                    )
                e_sbs[key] = e

            # ctx in [i-partitions, e-free] layout: per (pair, c) two psum
            # tiles of [128, 260], each holding two i-slices x two heads of
            # [128, 65] accumulation regions (64 ctx cols + denominator l in
            # col 64 via the V ones column).  stationary = e slice
            # [128 j, 128 i], moving = V [128 j, 65] -> 65-col matmuls, and
            # diagonal-trimmed i-slices are skipped outright.  The softmax
            # denominator lands per-partition, so normalization is a cheap
            # reciprocal + per-partition-scalar multiply straight into the
            # natural [s, e] output staging tile.  l-copies and muls run on
            # Pool so the DVE stays clear for the causal-mask muls; recip is
            # a DVE-only op but tiny.
            def finish_tile(tile_, pair, c, khalf):
                lsb = norm_pool.tile([128, 4], F32, tag="lsb", name="lsb")
                nc.vector.tensor_copy(
                    lsb.rearrange("p (g r) -> p g r", g=4),
                    tile_.rearrange("p (g r) -> p g r", r=65)[:, :, 64:65],
                )
                rec = norm_pool.tile([128, 4], F32, tag="rec", name="rec")
                nc.vector.reciprocal_approx_fast(out=rec[:], in_=lsb[:])
                for kloc in range(2):
                    k = 2 * khalf + kloc
                    for h2 in range(2):
                        reg = tile_[:, 130 * kloc + 65 * h2:
                                    130 * kloc + 65 * h2 + 64]
                        col = 256 * (4 * c + k) + 64 * (2 * pair + h2)
                        nc.vector.tensor_scalar_mul(
                            y_sb[:, col:col + 64],
                            reg,
                            rec[:, 2 * kloc + h2:2 * kloc + h2 + 1],
                        )

            # chunk 0: emit only what pair 0 needs up front (m=0 QK, V
            # tiles 0-3); the m=1 QK groups interleave into pair 0's keys so
            # the first exp lands ~5us earlier.
            pu0 = proj_units(0)
            for u in [pu0[0], pu0[2], pu0[1], pu0[3], pu0[5], pu0[7]]:
                u()  # qk(m0,q), qk(m0,k), v(t0..t3)
            carry = [pu0[4], pu0[6]]  # qk(m1,q), qk(m1,k)
            for c in range(NCH):
                units = carry + (proj_units(c + 1) if c + 1 < NCH else [])
                carry = []
                emitted = 0
                nkeys = 8 * (c + 1)
                kidx = 0
                njt = 4 * (c + 1)
                for pair in range(2):
                    keys = [(pair, c, jt) for jt in range(njt)]
                    emit_scores(keys[0])
                    ctxK = [
                        ctx_ps_pool.tile([128, 260], F32, tag="ctx",
                                         name="ctx_k01"),
                        ctx_ps_pool.tile([128, 260], F32, tag="ctx",
                                         name="ctx_k23"),
                    ]
                    for idx, key in enumerate(keys):
                        if idx + 1 < len(keys):
                            emit_scores(keys[idx + 1])
                        _, _, jt = key
                        r = max(0, jt - 4 * c)
                        e = e_sbs.pop(key)
                        # psum zero-regions are bank-sized (2KB): exactly ONE
                        # start per ctxK tile (its first matmul marks the
                        # whole bank pending-zero; every region's first touch
                        # at jt=0 then writes-not-accumulates), one stop on
                        # the tile's last matmul; group check off since the
                        # 8 regions share the bank's zero region.
                        for k in range(r, 4):
                            for h2 in range(2):
                                nc.tensor.matmul(
                                    ctxK[k // 2][:, 130 * (k % 2) + 65 * h2:
                                                 130 * (k % 2) + 65 * h2 + 65],
                                    e[:, 512 * h2 + 128 * k:
                                      512 * h2 + 128 * (k + 1)],
                                    vs(jt)[:, (DH + 1) * (2 * pair + h2):
                                           (DH + 1) * (2 * pair + h2 + 1)],
                                    start=(jt == 0 and h2 == 0
                                           and k in (0, 2)),
                                    stop=(jt == 4 * c + k and h2 == 1
                                          and k in (1, 3)),
                                    skip_group_check=True,
                                )
                        if jt == 4 * c + 1:
                            # ctxK[0]'s last contribution just stopped
                            finish_tile(ctxK[0], pair, c, 0)
                        # spread next chunk's projection groups across the
                        # attention stream so the PE keeps the ACT engine fed
                        # instead of batching projections between chunks
                        kidx += 1
                        want = len(units) * kidx // nkeys
                        while emitted < want:
                            units[emitted]()
                            emitted += 1
                    finish_tile(ctxK[1], pair, c, 1)
                    if pair == 1:
                        for khalf in range(2):
                            t0 = 4 * c + 2 * khalf
                            nc.sync.dma_start(
                                out=y[t0:t0 + 2].rearrange("t p e -> p t e"),
                                in_=y_sb[:, 256 * t0:256 * (t0 + 2)].rearrange(
                                    "p (t e) -> p t e", t=2),
                            )
                assert emitted == len(units)
    nc.compile()
    return nc


_CACHED = None


def get_nc():
    global _CACHED
    if _CACHED is None:
        _CACHED = build_kernel()
    return _CACHED


def _make_mask():
    """[128, 512] bf16: cols [128,256) and [384,512) hold the boundary
    mask M[dj, i''] = 1 if i'' >= dj (keep at-or-below diagonal)."""
    import ml_dtypes
    m = np.zeros((128, 512), np.float32)
    tri = np.triu(np.ones((128, 128), np.float32))  # tri[dj, i''] = i'' >= dj
    m[:, 128:256] = tri
    m[:, 384:512] = tri
    return m.astype(ml_dtypes.bfloat16)


def shard_inputs(x, W_query, W_key, W_value):
    """Full inputs -> per-core input maps (bf16 on the wire)."""
    import ml_dtypes
    bf = ml_dtypes.bfloat16
    in_maps = []
    ones = np.ones((128, 4, HPC, 1), bf)
    mask = _make_mask()
    # one transpose per batch, shared by the 4 cores of that batch
    xT_by_batch = [np.ascontiguousarray(x[b].T.astype(bf)) for b in range(2)]
    Wq16 = W_query.astype(bf)
    Wk16 = W_key.astype(bf)
    Wv16 = W_value.astype(bf)
    for core in range(8):
        b, g = core // 4, core % 4
        sl = slice(256 * g, 256 * (g + 1))
        in_maps.append({
            "xT": xT_by_batch[b],
            "wq": np.ascontiguousarray(Wq16[:, sl]),
            "wk": np.ascontiguousarray(Wk16[:, sl]),
            "wv": np.ascontiguousarray(Wv16[:, sl]),
            "ones_in": ones,
            "mask_in": mask,
        })
    return in_maps


def assemble_output(results):
    """Per-core y [16, 128, 256] (natural [s, e] tiles) -> full [2, S, 1024]."""
    out = np.empty((2, S, 1024), np.float32)
    for core in range(8):
        b, g = core // 4, core % 4
        yv = results[core]["y"]  # [16, 128, 256]
        out[b, :, 256 * g:256 * (g + 1)] = yv.reshape(S, 256)
    return out


def kernel(x, W_query, W_key, W_value):
    """Full inputs in, full output out; 8-core SPMD underneath."""
    from concourse.bass_utils import run_bass_kernel_spmd

    x = np.ascontiguousarray(np.asarray(x, dtype=np.float32))
    W_query = np.ascontiguousarray(np.asarray(W_query, dtype=np.float32))
    W_key = np.ascontiguousarray(np.asarray(W_key, dtype=np.float32))
    W_value = np.ascontiguousarray(np.asarray(W_value, dtype=np.float32))

    nc = get_nc()
    in_maps = shard_inputs(x, W_query, W_key, W_value)
    last_err = None
    for _attempt in range(3):
        try:
            res = run_bass_kernel_spmd(nc, in_maps, core_ids=list(range(8)))
            return assemble_output(res.results)
        except Exception as e:  # transient device wedges seen on this fabric
            last_err = e
            import time as _time
            _time.sleep(2.0)
    raise last_err
